# revision 37
# baseline (speedup 1.0000x reference)
"""RGCN (2x hetero GraphConv + mean-pool + MLP) on 8 TRN2 NeuronCores.

Sharding: nodes are dst-sharded 12500/core. Each core owns the aggregation
for its dst rows. Per-edge work is gather (bf16 rows from a replicated
node-feature table in local DRAM) -> scale by the folded degree norm
w_e = rsqrt(deg_in[dst]) * rsqrt(deg_out[src]) -> dma_scatter_add (CCE)
into SBUF accumulators.

The SWDGE descriptor generation for gather/scatter runs on one Q7 core pair
per queue (pair = queue_num); with num_swdge_queues=4 the four relations'
edge streams generate descriptors concurrently on four core pairs. Each
relation owns its own accumulator pair (no cross-relation merge); the W_r
pass accumulates the four relations directly in PSUM. Per-chunk edge
scaling runs on the Scalar engine (the Vector engine stalls on SBUF port
contention while Q7 descgen is active).

Layer-1 output blocks are relu'd, transposed and stored to a DRAM shard,
then AllGathered so every core has the full table for layer-2 gathers.
Layer 2 never materializes its output: pooling is re-associated as
K_r = agg_r^T @ gmat (lhsT=agg, no transpose needed) and
pooled = sum_r W_r^T @ K_r, then AllReduce and the tiny MLP head.

The instruction stream is identical on all 8 cores (SPMD); all per-core
variation lives in input tensors (gather/scatter indices, edge weights,
graph assignment). Host-side numpy only computes graph-structure metadata
(degrees/index layouts) and dtype/layout staging of inputs.
"""

import numpy as np
from ml_dtypes import bfloat16

import concourse.bass as bass
import concourse.bacc as bacc
import concourse.mybir as mybir
import concourse.tile as tile
from concourse import bass_utils
from concourse.masks import make_identity

F32 = mybir.dt.float32
BF16 = mybir.dt.bfloat16
I16 = mybir.dt.int16

# problem constants (hardcoded per spec)
N, E, NREL, G, IN, H, C = 100000, 400000, 4, 64, 64, 128, 2
CORES = 8
SHARD = N // CORES            # 12500
NBLK = (SHARD + 127) // 128   # 98
GRP = 25000                   # gather table rows per src-group (int16 idx limit)
NGRP = N // GRP               # 4
CALL = 2048                   # max gather/scatter indices per SWDGE call
# pad scatters land in a dedicated extra accum block at slot NBLK*128


# ---------------------------------------------------------------------------
# host-side planning: pure graph-structure metadata (indices, degrees, layout)
# ---------------------------------------------------------------------------

def _plan(src, dst, graph_ids):
    src = np.asarray(src).astype(np.int64)
    dst = np.asarray(dst).astype(np.int64)
    gid = np.asarray(graph_ids).astype(np.int64)

    # folded normalization: w_e = rsqrt(deg_in[dst]) * rsqrt(deg_out[src])
    w_all = np.empty((NREL, E), np.float32)
    for r in range(NREL):
        do = np.maximum(np.bincount(src[r], minlength=N), 1.0)
        di = np.maximum(np.bincount(dst[r], minlength=N), 1.0)
        w_all[r] = (1.0 / np.sqrt(do[src[r]]) / np.sqrt(di[dst[r]])).astype(np.float32)

    # dma_scatter_add races on duplicate indices within one call, so edges are
    # split into rounds: round k holds the k-th edge of each dst node. One
    # scatter call never spans a round boundary -> indices unique per call.
    rounds_all = {}
    nrounds_max = 0
    for c in range(CORES):
        for r in range(NREL):
            in_core = (dst[r] // SHARD) == c
            for g in range(NGRP):
                sel = np.nonzero(in_core & ((src[r] // GRP) == g))[0]
                order = np.argsort(dst[r][sel], kind="stable")
                sel = sel[order]
                dloc = dst[r][sel] - c * SHARD
                # rank of each edge within its dst group = round index
                if sel.size:
                    first = np.ones(sel.size, np.int64)
                    first[1:] = (np.diff(dloc) != 0).astype(np.int64)
                    run_start = np.nonzero(first)[0]
                    rank = np.arange(sel.size) - np.repeat(
                        run_start, np.diff(np.append(run_start, sel.size)))
                    nr = int(rank.max()) + 1
                    rounds = [sel[rank == k] for k in range(nr)]
                else:
                    rounds = []
                rounds_all[(c, r, g)] = rounds
                nrounds_max = max(nrounds_max, len(rounds))

    # SPMD-uniform round sizes: global max per round index, rounded to 128
    RSZ = []
    for k in range(nrounds_max):
        m = max(len(rounds_all[key][k]) if len(rounds_all[key]) > k else 0
                for key in rounds_all)
        RSZ.append(-(-m // 128) * 128)
    LRG = sum(RSZ)
    ncol = LRG // 16                            # idx columns per run
    nch = LRG // 128                            # chunks per run
    runs = NREL * NGRP
    roff = np.concatenate([[0], np.cumsum(RSZ)]).astype(np.int64)

    gidx = np.zeros((CORES, 16, runs * ncol), np.int16)
    didx = np.full((CORES, 16, runs * ncol), NBLK * 128, np.int16)
    wmeta = np.zeros((CORES, 128, runs * nch), np.float32)

    for c in range(CORES):
        for r in range(NREL):
            for g in range(NGRP):
                run = r * NGRP + g
                rounds = rounds_all[(c, r, g)]
                for k, e in enumerate(rounds):
                    kk = len(e)
                    if kk == 0:
                        continue
                    pos = roff[k] + np.arange(kk)
                    gi = (src[r][e] - (src[r][e] // GRP) * GRP).astype(np.int16)
                    di_ = (dst[r][e] - c * SHARD).astype(np.int16)
                    gidx[c, pos % 16, run * ncol + pos // 16] = gi
                    didx[c, pos % 16, run * ncol + pos // 16] = di_
                    wmeta[c, pos % 128, run * nch + pos // 128] = w_all[r][e]

    # graph assignment matrix with 1/count folded in
    cnt = np.maximum(np.bincount(gid, minlength=G), 1.0)
    gmat = np.zeros((CORES, NBLK * 128, G), np.float32)
    for c in range(CORES):
        ids = gid[c * SHARD:(c + 1) * SHARD]
        gmat[c, np.arange(SHARD), ids] = 1.0 / cnt[ids]

    # gather/scatter windows of <= CALL idxs, cut at BOTH the CALL grid and
    # round boundaries. One scatter call per window: indices are unique
    # within a call (single round), and the per-queue ring order G,S,G,S...
    # separates any two different-round scatters (which share dst addresses)
    # by a full gather's descriptors in every SDMA engine FIFO, so their CCE
    # read-modify-writes never overlap.
    windows = []
    off = 0
    while off < LRG:
        k = int(np.searchsorted(roff, off, "right")) - 1
        b = min(off + CALL, int(roff[k + 1]))
        windows.append((off, b - off))
        off = b

    # pack gather+scatter idx columns per (run, window) so one DMA loads both
    gdidx = np.zeros((CORES, 16, runs * 2 * ncol), np.int16)
    for run in range(runs):
        for (off, s) in windows:
            p0 = run * 2 * ncol + 2 * off // 16
            gdidx[:, :, p0:p0 + s // 16] = \
                gidx[:, :, run * ncol + off // 16:run * ncol + (off + s) // 16]
            gdidx[:, :, p0 + s // 16:p0 + 2 * s // 16] = \
                didx[:, :, run * ncol + off // 16:run * ncol + (off + s) // 16]

    # idx tiles span 128 partitions: the 16-row wrap replicated for 8 Q7 cores
    gdidx = np.tile(gdidx, (1, 8, 1))
    return dict(LRG=LRG, ncol=ncol, nch=nch, windows=windows,
                gdidx=gdidx, wmeta=wmeta,
                gmat=gmat.astype(bfloat16))


# ---------------------------------------------------------------------------
# device program
# ---------------------------------------------------------------------------

def _build(plan):
    ncol = plan["ncol"]
    nch = plan["nch"]
    windows = plan["windows"]
    runs = NREL * NGRP
    TCH = runs * nch

    nc = bacc.Bacc(None, target_bir_lowering=False, num_devices=CORES,
                   num_swdge_queues=4)

    # kernel I/O
    p = {}
    p["xT"] = nc.declare_dram_parameter("xT", [IN + 1, N], BF16, isOutput=False)
    p["W65"] = nc.declare_dram_parameter("W65", [IN + 1, H], BF16, isOutput=False)
    p["Wl1"] = nc.declare_dram_parameter("Wl1", [NREL, H, H], BF16, isOutput=False)
    p["Wl2"] = nc.declare_dram_parameter("Wl2", [NREL, H, H], BF16, isOutput=False)
    p["Wm1"] = nc.declare_dram_parameter("Wm1", [H, H], BF16, isOutput=False)
    p["Wm2"] = nc.declare_dram_parameter("Wm2", [H, H], BF16, isOutput=False)
    p["Wm3"] = nc.declare_dram_parameter("Wm3", [H, C], BF16, isOutput=False)
    p["B1"] = nc.declare_dram_parameter("B1", [H, 1], F32, isOutput=False)
    p["B2"] = nc.declare_dram_parameter("B2", [H, 1], F32, isOutput=False)
    p["bm1"] = nc.declare_dram_parameter("bm1", [H, 1], F32, isOutput=False)
    p["bm2"] = nc.declare_dram_parameter("bm2", [H, 1], F32, isOutput=False)
    p["bm3"] = nc.declare_dram_parameter("bm3", [C, 1], F32, isOutput=False)
    p["gdidx"] = nc.declare_dram_parameter("gdidx", [128, runs * 2 * ncol], I16,
                                           isOutput=False)
    p["wmeta"] = nc.declare_dram_parameter("wmeta", [128, TCH], BF16, isOutput=False)
    p["gmat"] = nc.declare_dram_parameter("gmat", [NBLK * 128, G], BF16, isOutput=False)
    out_ext = nc.declare_dram_parameter("out", [C, G], F32, isOutput=True)

    # internal DRAM
    h0_g = [nc.dram_tensor(f"h0_g{g}", [GRP, H], BF16) for g in range(NGRP)]
    h1_shard = nc.dram_tensor("h1_shard", [SHARD, H], BF16)
    h1_full = nc.dram_tensor("h1_full", [N, H], BF16, addr_space="Shared")
    pool_in = nc.dram_tensor("pool_in", [H, G], F32)
    pool_out = nc.dram_tensor("pool_out", [H, G], F32, addr_space="Shared")

    rg = [list(range(CORES))]

    with tile.TileContext(nc) as tc:
        with (
            tc.tile_pool(name="const", bufs=1) as cpool,
            tc.tile_pool(name="meta", bufs=1) as mpool,
            tc.tile_pool(name="stage", bufs=3) as spool,
            tc.tile_pool(name="idx", bufs=5) as ipool,
            tc.tile_pool(name="accum", bufs=1) as apool,
            tc.tile_pool(name="work", bufs=4) as wpool,
            tc.tile_pool(name="h0s", bufs=2) as hpool,
            tc.tile_pool(name="po", bufs=2, space="PSUM") as po,       # W_r matmul accum
            tc.tile_pool(name="pbw", bufs=3, space="PSUM") as pbw,     # W_r transposes
            tc.tile_pool(name="pbt", bufs=2, space="PSUM") as pbt,     # epilogue transpose
            tc.tile_pool(name="pk", bufs=1, space="PSUM") as pk,       # layer-2 K accum
        ):
            # ---- constants into SBUF
            id_f32 = cpool.tile([128, 128], F32)
            make_identity(nc, id_f32[:])
            id_bf = cpool.tile([128, 128], BF16)
            nc.vector.tensor_copy(id_bf[:], id_f32[:])

            w65 = cpool.tile([IN + 1, H], BF16)
            nc.sync.dma_start(w65[:], p["W65"][:, :])
            wl = {}
            for li, name in ((1, "Wl1"), (2, "Wl2")):
                for r in range(NREL):
                    t = cpool.tile([H, H], BF16, tag=f"wl{li}{r}")
                    nc.sync.dma_start(t[:], p[name][r, :, :])
                    wl[(li, r)] = t
            wm = {}
            for name in ("Wm1", "Wm2"):
                t = cpool.tile([H, H], BF16, tag=name)
                nc.sync.dma_start(t[:], p[name][:, :])
                wm[name] = t
            wm3 = cpool.tile([H, C], BF16)
            nc.sync.dma_start(wm3[:], p["Wm3"][:, :])
            biases = {}
            for name in ("B1", "B2", "bm1", "bm2"):
                t = cpool.tile([H, 1], F32, tag=name)
                nc.sync.dma_start(t[:], p[name][:, :])
                biases[name] = t
            bm3 = cpool.tile([C, 1], F32)
            nc.sync.dma_start(bm3[:], p["bm3"][:, :])

            wmeta = mpool.tile([128, TCH], BF16)
            nc.sync.dma_start(wmeta[:], p["wmeta"][:, :])



            # per-relation accumulators (parity-split pairs), shared by layers
            accs = [(apool.tile([128, (NBLK + 2) // 2, H], BF16,
                                name=f"ae{r}", tag=f"ae{r}"),
                     apool.tile([128, (NBLK + 2) // 2, H], BF16,
                                name=f"ao{r}", tag=f"ao{r}"))
                    for r in range(NREL)]

            # ---- phase 0: h0 = relu(x @ W_in + b_in), node-major, replicated
            STRIP = 2048
            n_strip = -(-N // STRIP)
            ti = 0
            for s in range(n_strip):
                w = min(STRIP, N - s * STRIP)
                strip = hpool.tile([IN + 1, STRIP], BF16, tag="h0strip")
                nc.sync.dma_start(strip[:, :w], p["xT"][:, s * STRIP:s * STRIP + w])
                for q0 in range(0, w, 1024):
                    qw = min(1024, w - q0)
                    nt = -(-qw // 128)
                    hb = wpool.tile([128, 8, H], BF16, tag="h0out")
                    for t in range(nt):
                        t0 = q0 + t * 128
                        tw = min(128, w - t0)
                        ps = po.tile([128, H], F32, tag="mm")
                        nc.tensor.matmul(ps[:tw, :], lhsT=strip[:, t0:t0 + tw],
                                         rhs=w65[:], start=True, stop=True)
                        nc.vector.tensor_scalar_max(
                            hb[:tw, t, :], ps[:tw, :], 0.0)
                        ti += 1
                    lo = s * STRIP + q0
                    if qw == 1024 and lo // GRP == (lo + qw - 1) // GRP:
                        g0 = lo // GRP
                        a = lo - g0 * GRP
                        nc.sync.dma_start(
                            h0_g[g0][a:a + qw, :].rearrange(
                                "(t p) f -> p t f", p=128),
                            hb[:, :8, :])
                    else:
                        for t in range(nt):
                            t0 = lo + t * 128
                            tw = min(128, s * STRIP + w - t0)
                            done = 0
                            while done < tw:
                                g0 = (t0 + done) // GRP
                                take = min(tw - done,
                                           (g0 + 1) * GRP - (t0 + done))
                                nc.sync.dma_start(
                                    h0_g[g0][t0 + done - g0 * GRP:
                                             t0 + done - g0 * GRP + take, :],
                                    hb[done:done + take, t, :])
                                done += take

            # ---- conv layers
            LAG = 2

            def edge_phase(tables):
                """gather -> scale -> scatter-add for all 4 relations, each on
                its own SWDGE queue (= Q7 core pair) and accumulator pair.
                Scatters lag gathers by LAG steps so the Scalar scale chain
                never stalls the Pool stream."""
                for r in range(NREL):
                    nc.vector.memset(accs[r][0][:], 0.0)
                    nc.vector.memset(accs[r][1][:], 0.0)

                steps = [(g, wi) for g in range(NGRP)
                         for wi in range(len(windows))]
                pend = {}  # si -> {r: (st, gd, S)}
                for si in range(len(steps) + LAG):
                    if si < len(steps):
                        g, wi = steps[si]
                        off, S = windows[wi]
                        ch = S // 128
                        nxt = {}
                        for r in range(NREL):
                            run = r * NGRP + g
                            p0 = run * 2 * ncol + 2 * off // 16
                            gd = ipool.tile([128, 2 * CALL // 16], I16,
                                            tag=f"gd{r}")
                            nc.sync.dma_start(
                                gd[:, :2 * S // 16],
                                p["gdidx"][:, p0:p0 + 2 * S // 16])
                            st = spool.tile([128, CALL // 128, H], BF16,
                                            tag=f"st{r}")
                            nc.gpsimd.dma_gather(
                                st[:, :ch, :], tables[g][:, :],
                                gd[:, :S // 16], S, S, H,
                                single_packet=False, queue_num=r)
                            nxt[r] = (st, gd, run * nch + off // 128, S)
                        # scale in place: ONE broadcast multiply per window
                        # (weight column stretched along the feature axis)
                        for r in range(NREL):
                            st, gd, ch0, S_ = nxt[r]
                            ch_n = S_ // 128
                            nc.vector.tensor_mul(
                                st[:, :ch_n, :], st[:, :ch_n, :],
                                wmeta[:, ch0:ch0 + ch_n, None].broadcast_to(
                                    [128, ch_n, H]))
                        pend[si] = nxt
                    if si >= LAG:
                        for r in range(NREL):
                            st, gd, ch0, S_ = pend[si - LAG][r]
                            ae, ao = accs[r]
                            nc.gpsimd.dma_scatter_add(
                                ae[:, :, :], st[:, :S_ // 128, :],
                                gd[:, S_ // 16:2 * S_ // 16], S_, S_, H,
                                sbuf_tokens_per_rank=128, parity_reg=0,
                                out_ap_other=ao[:, :, :],
                                single_packet=False, queue_num=r)
                        del pend[si - LAG]

            # ----- layer 1
            edge_phase(h0_g)
            # W_r pass: accumulate the 4 relations in PSUM per block, then
            # relu+bias, transpose back to node-major, store the shard.
            for b in range(NBLK):
                o2 = po.tile([128, H], F32, tag="mm")
                aggTs = []
                for r in range(NREL):
                    tp = pbw.tile([128, 128], BF16, tag="wtp")
                    nc.tensor.transpose(tp[:], accs[r][b % 2][:, b // 2, :],
                                        id_bf[:])
                    aggT = wpool.tile([128, 128], BF16, tag="aggT")
                    nc.vector.tensor_copy(aggT[:], tp[:])
                    aggTs.append(aggT)
                for r in range(NREL):
                    nc.tensor.matmul(o2[:], lhsT=wl[(1, r)][:], rhs=aggTs[r][:],
                                     start=(r == 0), stop=(r == NREL - 1))
                rows = min(128, SHARD - b * 128)
                t1 = wpool.tile([128, 128], BF16, tag="t1")
                nc.scalar.activation(t1[:], o2[:],
                                     mybir.ActivationFunctionType.Relu,
                                     bias=biases["B1"][:, :])
                tb = pbt.tile([128, 128], BF16, tag="tb")
                nc.tensor.transpose(tb[:], t1[:], id_bf[:])
                t2 = wpool.tile([128, 128], BF16, tag="t2")
                nc.vector.tensor_copy(t2[:], tb[:])
                nc.sync.dma_start(h1_shard[b * 128:b * 128 + rows, :],
                                  t2[:rows, :])
            nc.gpsimd.collective_compute(
                "AllGather", mybir.AluOpType.bypass, replica_groups=rg,
                ins=[h1_shard[:, :]], outs=[h1_full[:, :]])

            # ----- layer 2
            edge_phase([h1_full[g * GRP:(g + 1) * GRP, :] for g in range(NGRP)])
            # pooling, re-associated: K_r[i, G] = sum_n agg_r[n, i] gmat[n, G]
            # (lhsT = agg block directly; no transpose), then
            # pooled[o, G] = sum_r sum_i W_r[i, o] K_r[i, G].
            # one PSUM accumulation chain per relation, each in its own po
            # buffer (chains sharing a PSUM bank corrupt each other's
            # start/stop), gmat blocks held in SBUF across the 4 passes
            gms = []
            for b in range(NBLK):
                rows = min(128, SHARD - b * 128)
                gm = cpool.tile([128, G], BF16, name=f"gmb{b}", tag=f"gmb{b}")
                nc.sync.dma_start(
                    gm[:rows, :], p["gmat"][b * 128:b * 128 + rows, :])
                gms.append(gm)
            kss = []
            for r in range(NREL):
                kt = po.tile([H, G], F32, tag="mm")
                for b in range(NBLK):
                    rows = min(128, SHARD - b * 128)
                    nc.tensor.matmul(kt[:],
                                     lhsT=accs[r][b % 2][:rows, b // 2, :],
                                     rhs=gms[b][:rows, :],
                                     start=(b == 0), stop=(b == NBLK - 1))
                ks = wpool.tile([H, G], BF16, tag=f"ks{r}")
                nc.vector.tensor_copy(ks[:], kt[:])
                kss.append(ks)
            pooled = po.tile([H, G], F32, tag="mm")
            for r in range(NREL):
                nc.tensor.matmul(pooled[:], lhsT=wl[(2, r)][:], rhs=kss[r][:],
                                 start=(r == 0), stop=(r == NREL - 1))

            # ---- pooled partial sums -> AllReduce -> head
            psb = wpool.tile([H, G], F32, tag="psb")
            nc.scalar.activation(psb[:], pooled[:],
                                 mybir.ActivationFunctionType.Copy)
            nc.sync.dma_start(pool_in[:, :], psb[:])
            nc.gpsimd.collective_compute(
                "AllReduce", mybir.AluOpType.add, replica_groups=rg,
                ins=[pool_in[:, :]], outs=[pool_out[:, :]])
            pool_f = wpool.tile([H, G], F32, tag="pool_f")
            nc.sync.dma_start(pool_f[:], pool_out[:, :])
            # + B2 (conv2 bias, linear through the mean), cast to bf16
            pool_b = wpool.tile([H, G], BF16, tag="pool_b")
            nc.vector.tensor_scalar_add(pool_b[:], pool_f[:], biases["B2"][:, :])

            z1p = po.tile([H, G], F32, tag="mm")
            nc.tensor.matmul(z1p[:], lhsT=wm["Wm1"][:], rhs=pool_b[:],
                             start=True, stop=True)
            z1 = wpool.tile([H, G], BF16, tag="z1")
            nc.scalar.activation(z1[:], z1p[:],
                                 mybir.ActivationFunctionType.Relu,
                                 bias=biases["bm1"][:, :])
            z2p = po.tile([H, G], F32, tag="mm")
            nc.tensor.matmul(z2p[:], lhsT=wm["Wm2"][:], rhs=z1[:],
                             start=True, stop=True)
            z2 = wpool.tile([H, G], BF16, tag="z2")
            nc.scalar.activation(z2[:], z2p[:],
                                 mybir.ActivationFunctionType.Relu,
                                 bias=biases["bm2"][:, :])
            z3p = po.tile([C, G], F32, tag="mm")
            nc.tensor.matmul(z3p[:], lhsT=wm3[:], rhs=z2[:],
                             start=True, stop=True)
            z3 = wpool.tile([C, G], F32, tag="z3")
            nc.vector.tensor_scalar_add(z3[:], z3p[:], bm3[:, :])
            nc.sync.dma_start(out_ext[:, :], z3[:])

    nc.compile()
    return nc


# ---------------------------------------------------------------------------
# entry point
# ---------------------------------------------------------------------------

_CACHE = {}


def kernel(x, src, dst, graph_ids, W_in, b_in, W1, b1, W2, b2,
           Wm1, bm1, Wm2, bm2, Wm3, bm3):
    x = np.asarray(x)
    key = (int(np.asarray(src).sum()) ^ int(np.asarray(dst).sum()),
           int(np.asarray(graph_ids).sum()))
    if key not in _CACHE:
        plan = _plan(src, dst, graph_ids)
        nc = _build(plan)
        _CACHE[key] = (plan, nc)
    plan, nc = _CACHE[key]

    xT = np.concatenate([np.asarray(x).T, np.ones((1, N), np.float32)], axis=0)
    w65 = np.concatenate([np.asarray(W_in), np.asarray(b_in)[None, :]], axis=0)

    def bf(a):
        return np.ascontiguousarray(np.asarray(a), dtype=np.float32).astype(bfloat16)

    def col(a):
        return np.ascontiguousarray(np.asarray(a, np.float32).reshape(-1, 1))

    in_maps = []
    for c in range(CORES):
        in_maps.append({
            "xT": bf(xT),
            "W65": bf(w65),
            "Wl1": bf(W1),
            "Wl2": bf(W2),
            "Wm1": bf(Wm1),
            "Wm2": bf(Wm2),
            "Wm3": bf(Wm3),
            "B1": col(np.asarray(b1, np.float32).sum(axis=0)),
            "B2": col(np.asarray(b2, np.float32).sum(axis=0)),
            "bm1": col(bm1),
            "bm2": col(bm2),
            "bm3": col(bm3),
            "gdidx": np.ascontiguousarray(plan["gdidx"][c]),
            "wmeta": np.ascontiguousarray(plan["wmeta"][c]).astype(bfloat16),
            "gmat": np.ascontiguousarray(plan["gmat"][c]),
        })

    res = bass_utils.run_bass_kernel_spmd(nc, in_maps, list(range(CORES)))
    global LAST_EXEC_NS
    LAST_EXEC_NS = res.exec_time_ns
    out = np.asarray(res.results[0]["out"], np.float32)  # [C, G]
    return np.ascontiguousarray(out.T)                   # [G, C]


LAST_EXEC_NS = None


if __name__ == "__main__":
    import reference
    import jax
    with jax.default_device(jax.devices("cpu")[0]):
        inp = {k: np.asarray(v) for k, v in reference.setup_inputs().items()}
        exp = np.asarray(reference.reference(**{k: v for k, v in inp.items()}))
    act = kernel(**inp)
    rel = np.linalg.norm(act - exp) / np.linalg.norm(exp)
    print("Relative error:", rel)
